# revision 26
# baseline (speedup 1.0000x reference)
"""Trainium2 Bass kernel for causal GQA attention block (dense transformer).

Full module: qkv = clip(x @ Wqkv.T, +-8); half-split RoPE on q,k;
GQA causal attention (32 q heads, 8 kv heads, head_dim 128); out @ Wout.T.

Sharding: tensor-parallel over heads across 8 cores. Each core owns 4 q
heads + their shared kv head (rows of Wqkv) and the matching 512 columns
of Wout; it computes a full-shape partial of the output projection and the
host sums the 8 partials.

Final version (measured 792835ns vs 869332ns baseline, rel2 5.8e-3):
- RoPE rotation via SBUF->SBUF half-swap DMAs on the (qkv-phase-idle)
  gpsimd queue + vector multiply-adds - no PE involvement.
- Both batches' qkv projections run first as one dense PE block; the
  attention chunks of the two batches are then interleaved so every
  chunk has independent chains + the previous chunk's out-projection
  as PE filler (deferred per row-block, interleaved after each head).
- Softmax denominator via one all-ones [128,128] stationary matmul
  (broadcasts the k-sum to all partitions); reciprocal feeds the
  normalization multiply directly.
- Diag tiles: single full-width exp + 0/1 triangular zeroing multiply.
- Outputs stored as bf16 partials, coalesced into [128,2048] stores.
- Startup: all 32 (w_ci, x_ci) tile pairs of the first l-chunk are
  prefetched in strict need-order across three DMA queues; Wout and
  the remaining rope tables are deferred behind the x traffic.
- qkv-phase SBUF pools are scoped and released before the attention
  pools open (both batches' q/k/v tiles need 48KB/partition).
"""
import os
import sys
import math

for _p in ("/opt/trn_rl_repo", "/root/.axon_site/_ro/trn_rl_repo"):
    if os.path.isdir(_p) and _p not in sys.path:
        sys.path.insert(0, _p)

import numpy as np

import concourse.bass as bass
import concourse.tile as tile
from concourse import bacc, mybir
from concourse import bass_utils

# If BASS_TRACE is set in the environment, run_bass_kernel_spmd imports
# antenv.axon_hooks, which this image's antenv package lacks. Register a
# stub so tracing degrades gracefully instead of crashing.
try:
    import antenv.axon_hooks  # noqa: F401
except ImportError:
    try:
        import types
        import antenv

        _hooks = types.ModuleType("antenv.axon_hooks")
        _hooks._hook = None
        _hooks.set_axon_ntff_profile_hook = (
            lambda h: setattr(_hooks, "_hook", h))
        _hooks.get_axon_ntff_profile_hook = lambda: _hooks._hook
        sys.modules["antenv.axon_hooks"] = _hooks
        antenv.axon_hooks = _hooks
    except Exception:  # noqa: BLE001
        pass

F32 = mybir.dt.float32
F32R = mybir.dt.float32r
BF16 = mybir.dt.bfloat16
AF = mybir.ActivationFunctionType
OP = mybir.AluOpType


def default_cfg():
    return dict(
        B=2, L=2048, D=4096, QH=4, HD=128,
        CH=512,   # attention q-chunk width
        QN=512,   # qkv projection l-chunk width
        OG=512,   # output projection column-group width
        CLIP=8.0, theta=500000.0, ncores=8,
    )


def mini_cfg():
    return dict(
        B=2, L=512, D=512, QH=2, HD=128,
        CH=256, QN=256, OG=256,
        CLIP=8.0, theta=500000.0, ncores=1,
    )


def build_program(cfg):
    B, L, D = cfg["B"], cfg["L"], cfg["D"]
    QH, HD = cfg["QH"], cfg["HD"]
    CH, QN, OG = cfg["CH"], cfg["QN"], cfg["OG"]
    CLIP = cfg["CLIP"]
    RT = QH + 2               # r-tiles per core: QH q heads, k, v
    R = RT * HD
    CT = D // HD              # contraction tiles
    TPC = CH // HD            # k-tiles per attention chunk
    NJ = L // CH              # attention q-chunks per batch
    NLC = L // QN             # qkv l-chunks per batch
    HF = HD // 2              # rope half
    scale = 1.0 / math.sqrt(HD)

    nc = bacc.Bacc("TRN2", target_bir_lowering=False, debug=False,
                   enable_asserts=True, num_devices=1)

    xT_d = nc.dram_tensor("xT", [D, B * L], BF16, kind="ExternalInput").ap()
    wq_d = nc.dram_tensor("wq", [D, R], BF16, kind="ExternalInput").ap()
    wo_d = nc.dram_tensor("wo", [QH * HD, D], BF16, kind="ExternalInput").ap()
    ra_d = nc.dram_tensor("ropeA", [HD, L], F32, kind="ExternalInput").ap()
    rb_d = nc.dram_tensor("ropeB", [HD, L], F32, kind="ExternalInput").ap()
    tr_d = nc.dram_tensor("tri01", [HD, HD], BF16, kind="ExternalInput").ap()
    pm_d = nc.dram_tensor("perm", [HD, HD], F32R, kind="ExternalInput").ap()
    o2_d = nc.dram_tensor("ones2", [HD, HD], F32R, kind="ExternalInput").ap()
    id_d = nc.dram_tensor("ident", [HD, HD], F32R, kind="ExternalInput").ap()
    out_d = nc.dram_tensor("out", [B * L, D], BF16, kind="ExternalOutput").ap()

    from contextlib import ExitStack
    with tile.TileContext(nc) as tc, ExitStack() as _es:
        cpool = _es.enter_context(tc.tile_pool(name="const", bufs=1))
        tb_pool = _es.enter_context(tc.tile_pool(name="tb", bufs=1))
        wo_pool = _es.enter_context(tc.tile_pool(name="wo", bufs=1))
        qkv_pool = _es.enter_context(tc.tile_pool(name="qkv", bufs=1))
        # qkv-phase-only pools live in their own scope so their SBUF
        # (~82KB/partition, dominated by the resident Wqkv tiles) is
        # released before the attention-phase pools open.
        _qes = ExitStack()
        wq_pool = _qes.enter_context(tc.tile_pool(name="wq_pool", bufs=1))
        x_pool = _qes.enter_context(tc.tile_pool(name="xp", bufs=6))
        xpre_pool = _qes.enter_context(tc.tile_pool(name="xpre", bufs=1))
        cl_pool = _qes.enter_context(tc.tile_pool(name="cl", bufs=1))
        rt_pool = _qes.enter_context(tc.tile_pool(name="rt", bufs=2))
        PA = _es.enter_context(tc.tile_pool(name="PA", bufs=2, space="PSUM"))
        PB = _es.enter_context(tc.tile_pool(name="PB", bufs=2, space="PSUM"))
        PC = _es.enter_context(tc.tile_pool(name="PC", bufs=2, space="PSUM"))
        PD = _es.enter_context(tc.tile_pool(name="PD", bufs=2, space="PSUM"))
        t_tri = cpool.tile([HD, HD], BF16, tag="t_tri")
        t_perm = cpool.tile([HD, HD], F32R, tag="t_perm")
        t_ones2 = cpool.tile([HD, HD], F32R, tag="t_ones2")
        t_ident = cpool.tile([HD, HD], F32R, tag="t_ident")

        # resident qkv weights (bf16), one tile per contraction block.
        # DMA order: strict need-order for the first accumulation chain -
        # (w_ci, x_ci) pairs round-robin over three queues.  Everything
        # the attention phase needs later (rope tables, tri mask, Wout)
        # follows on the gpsimd queue.
        w_ci = [wq_pool.tile([HD, R], BF16, tag=f"w{ci}", name=f"w{ci}")
                for ci in range(CT)]
        NPRE = CT
        xt_pre = [xpre_pool.tile([HD, QN], BF16, tag=f"xpre{ci}",
                                 name=f"xpre{ci}")
                  for ci in range(NPRE)]
        qs = [nc.sync, nc.scalar]
        q3 = [nc.sync, nc.scalar, nc.gpsimd]
        q3[0].dma_start(w_ci[0][:], wq_d[0:HD, :])
        q3[1].dma_start(xt_pre[0][:], xT_d[0:HD, 0:QN])
        for ci in range(1, CT):
            q3[ci % 3].dma_start(w_ci[ci][:], wq_d[ci * HD:(ci + 1) * HD, :])
            if ci < NPRE:
                q3[(ci + 1) % 3].dma_start(
                    xt_pre[ci][:], xT_d[ci * HD:(ci + 1) * HD, 0:QN])
        # rope tables: one load for both batches
        t_ra = [tb_pool.tile([HD, QN], F32, tag=f"t_ra{lc}",
                             name=f"t_ra{lc}")
                for lc in range(NLC)]
        t_rb = [tb_pool.tile([HD, QN], F32, tag=f"t_rb{lc}",
                             name=f"t_rb{lc}")
                for lc in range(NLC)]
        nc.gpsimd.dma_start(t_ra[0][:], ra_d[:, 0:QN])
        nc.gpsimd.dma_start(t_rb[0][:], rb_d[:, 0:QN])
        nc.gpsimd.dma_start(t_tri[:], tr_d[:])
        nc.gpsimd.dma_start(t_perm[:], pm_d[:])
        nc.gpsimd.dma_start(t_ones2[:], o2_d[:])
        nc.gpsimd.dma_start(t_ident[:], id_d[:])
        for lc in range(1, NLC):
            nc.gpsimd.dma_start(t_ra[lc][:], ra_d[:, lc * QN:(lc + 1) * QN])
            nc.gpsimd.dma_start(t_rb[lc][:], rb_d[:, lc * QN:(lc + 1) * QN])
        # Wout: loaded once for both batches; the DMAs are emitted after
        # b0/lc1 (below) so they queue behind the startup x traffic and
        # don't steal HBM bandwidth from the first projection chunks.
        wo_sb = wo_pool.tile([HD, QH * D], BF16, tag="wo_sb")

        # PSUM accumulator assignment for the qkv phase: r-tile -> pool.
        # The last l-chunk avoids PA so early attention scores (PA) can
        # start while the final accumulations drain.
        accpool = [PA, PA, PB, PB, PC, PC][:RT]
        accpool_last = [PB, PB, PC, PC, PA, PA][:RT]

        def emit_fin_lt(b, j, at_tiles, lt, tail=False):
            # one row-block (lt) of the deferred out-projection: 8 fn
            # accumulations + evacuation copies + coalesced stores
            fnpools = [PD, PA, PB, PC] if tail else [PD]
            NOC = D // OG
            GW = min(4, NOC) if not tail else min(2, NOC)
            row0 = b * L + j * CH + lt * HD
            fo = None
            for oc in range(NOC):
                if oc % GW == 0:
                    fo = fo_pool.tile([HD, 4 * OG], BF16, tag="fo")
                fn = fnpools[oc % len(fnpools)].tile(
                    [HD, OG], F32, tag="p", name=f"fn{b}_{j}_{lt}_{oc}")
                for i in range(QH):
                    nc.tensor.matmul(
                        fn[:],
                        at_tiles[i][:, lt * HD:(lt + 1) * HD],
                        wo_sb[:, i * D + oc * OG:
                              i * D + (oc + 1) * OG],
                        start=(i == 0), stop=(i == QH - 1))
                fslot = fo[:, (oc % GW) * OG:(oc % GW + 1) * OG]
                if oc % 2 == 0:
                    nc.vector.tensor_copy(fslot, fn[:])
                else:
                    nc.scalar.copy(fslot, fn[:])
                if oc % GW == GW - 1:
                    eng = (q3[(oc // GW) % 3] if tail else nc.gpsimd)
                    eng.dma_start(
                        out_d[row0:row0 + HD,
                              (oc - GW + 1) * OG:(oc + 1) * OG],
                        fo[:, :GW * OG])

        def emit_rope(lc, cl, dest_t):
            # dest = cl*cos + swap_halves(cl)*ropeB, where ropeB carries
            # -sin in the top half and +sin in the bottom.  The half swap
            # is an SBUF->SBUF DMA on the gpsimd queue (idle during the
            # qkv phase); no PE involvement.
            clsw = rt_pool.tile([HD, QN], F32, tag="clsw")
            nc.gpsimd.dma_start(clsw[0:HF, :], cl[HF:HD, :].bitcast(F32))
            nc.gpsimd.dma_start(clsw[HF:HD, :], cl[0:HF, :].bitcast(F32))
            t1 = rt_pool.tile([HD, QN], F32, tag="t1")
            nc.vector.tensor_tensor(t1[:], clsw[:], t_rb[lc][:], OP.mult)
            nc.vector.tensor_tensor(
                dest_t[:], cl[:].bitcast(F32), t_ra[lc][:], OP.mult)
            nc.vector.tensor_tensor(
                dest_t[:], dest_t[:].bitcast(BF16), t1[:], OP.add)

        q_t = [[[qkv_pool.tile([HD, QN], BF16, tag=f"q{b}_{h}_{lc}",
                               name=f"q{h}_{b}_{lc}")
                 for lc in range(NLC)]
                for h in range(QH)]
               for b in range(B)]
        k_t = [[qkv_pool.tile([HD, QN], BF16, tag=f"k{b}_{lc}",
                              name=f"k_{b}_{lc}")
                for lc in range(NLC)]
               for b in range(B)]
        v_t = [[qkv_pool.tile([HD, QN], BF16, tag=f"v{b}_{lc}",
                              name=f"v_{b}_{lc}")
                for lc in range(NLC)]
               for b in range(B)]

        # ---------------- qkv projection phase (both batches) ----------
        for b in range(B):
            for lc in range(NLC):
                last_lc = (b == B - 1 and lc == NLC - 1)
                apool = accpool_last if last_lc else accpool
                acc = [apool[r].tile([HD, QN], F32, tag="p",
                                     name=f"acc{b}_{lc}_{r}")
                       for r in range(RT)]
                for ci in range(CT):
                    if b == 0 and lc == 0 and ci < NPRE:
                        xt = xt_pre[ci]
                    else:
                        xt = x_pool.tile([HD, QN], BF16, tag="xt")
                        # lc1 of b0 rides the gpsimd queue, which is idle
                        # after the header tables while sync/scalar still
                        # stream the prefetched first chunk
                        eng = (nc.gpsimd if (b == 0 and lc == 1)
                               else qs[ci % 2])
                        eng.dma_start(
                            xt[:],
                            xT_d[ci * HD:(ci + 1) * HD,
                                 b * L + lc * QN:
                                 b * L + (lc + 1) * QN])
                    for r in range(RT):
                        nc.tensor.matmul(
                            acc[r][:],
                            w_ci[ci][:, r * HD:(r + 1) * HD],
                            xt[:],
                            start=(ci == 0), stop=(ci == CT - 1))
                cls = []
                for r in range(RT):
                    cl = cl_pool.tile([HD, QN], F32R, tag=f"cl{r}",
                                      name=f"cl{b}_{lc}_{r}")
                    nc.vector.tensor_scalar(
                        cl[:], acc[r][:], -CLIP, CLIP,
                        OP.max, OP.min)
                    cls.append(cl)
                if b == 0 and lc == NLC - 1:
                    for i in range(QH):
                        for hh in range(2):
                            qs[(i + hh) % 2].dma_start(
                                wo_sb[:, i * D + hh * (D // 2):
                                      i * D + (hh + 1) * (D // 2)],
                                wo_d[i * HD:(i + 1) * HD,
                                     hh * (D // 2):(hh + 1) * (D // 2)])
                for r in range(RT):
                    cl = cls[r]
                    if r < QH + 1:
                        dest_t = q_t[b][r][lc] if r < QH else k_t[b][lc]
                        emit_rope(lc, cl, dest_t)
                    else:
                        vtr = PD.tile([HD, QN], F32R, tag="p",
                                      name=f"vtr{b}_{lc}")
                        nt = QN // HD
                        for t in range(nt):
                            nc.tensor.matmul(
                                vtr[:, t * HD:(t + 1) * HD],
                                cl[:, t * HD:(t + 1) * HD],
                                t_ident[:],
                                is_transpose=True,
                                start=(t == 0), stop=(t == nt - 1))
                        nc.scalar.copy(v_t[b][lc][:], vtr[:].bitcast(F32))

        # ------- attention + out projection, chunks interleaved --------
        _qes.close()
        ex_pool = _es.enter_context(tc.tile_pool(name="ex", bufs=8))
        ax_pool = _es.enter_context(tc.tile_pool(name="ax", bufs=4))
        at_pool = _es.enter_context(tc.tile_pool(name="at", bufs=2 * QH + 2))
        bc_pool = _es.enter_context(tc.tile_pool(name="bc", bufs=3))
        fo_pool = _es.enter_context(tc.tile_pool(name="fo", bufs=4))
        prev = None
        jseq = ([NJ - 1, 1, 0] + list(range(2, NJ - 1))
                if NJ > 2 else list(range(NJ)))
        order = [(b, j) for j in jseq for b in range(B)]
        for (b, j) in order:
            at_tiles = []
            for h in range(QH):
                pv = PB.tile([HD, CH], F32, tag="p",
                             name=f"pv{b}_{j}_{h}")
                axs = ax_pool.tile([HD, CH], F32R, tag="axs",
                                   name=f"axs{b}_{j}_{h}")
                axs2 = (ax_pool.tile([HD, CH], F32R, tag="axs2",
                                     name=f"axs2{b}_{j}_{h}")
                        if j > 0 else None)
                nk = (j + 1) * TPC
                for ki in range(nk):
                    diag = ki >= j * TPC
                    w0 = (ki - j * TPC) * HD if diag else 0
                    W = CH - w0
                    klc, kof = divmod(ki * HD, QN)
                    sc = PA.tile([HD, CH], F32, tag="p",
                                  name=f"sc{b}_{j}_{h}_{ki}")
                    nc.tensor.matmul(
                        sc[:, :W],
                        k_t[b][klc][:, kof:kof + HD],
                        q_t[b][h][j][:, w0:w0 + W],
                        start=True, stop=True)
                    ex = ex_pool.tile([HD, CH], BF16, tag="ex")
                    nc.scalar.activation(
                        ex[:, :W], sc[:, :W], AF.Exp,
                        scale=scale)
                    if diag:
                        nc.vector.tensor_tensor(
                            ex[:, :HD], ex[:, :HD], t_tri[:],
                            OP.mult)
                    nc.tensor.matmul(
                        pv[:, w0:w0 + W],
                        v_t[b][klc][:, kof:kof + HD],
                        ex[:, :W],
                        start=(ki == 0), stop=(ki == nk - 1))
                    # exp'd-score accumulation for the denominator:
                    # two chains (axs/axs2), both on gpsimd - vector
                    # stays free for the out-projection evacuations.
                    if ki == 0:
                        nc.vector.tensor_copy(axs[:], ex[:])
                    elif j > 0 and ki == 1:
                        nc.vector.tensor_copy(axs2[:], ex[:])
                    elif j == 0 or ki % 2 == 0:
                        nc.gpsimd.tensor_tensor(
                            axs[:, w0:w0 + W],
                            axs[:, w0:w0 + W].bitcast(F32),
                            ex[:, :W], OP.add)
                    else:
                        nc.vector.tensor_tensor(
                            axs2[:, w0:w0 + W],
                            axs2[:, w0:w0 + W].bitcast(F32),
                            ex[:, :W], OP.add)
                smb = PC.tile([HD, CH], F32, tag="p",
                              name=f"smb{b}_{j}_{h}")
                nc.tensor.matmul(smb[:], t_ones2[:], axs[:],
                                 start=True, stop=(j == 0))
                if j > 0:
                    nc.tensor.matmul(smb[:], t_ones2[:], axs2[:],
                                     start=False, stop=True)
                bcs = bc_pool.tile([HD, CH], F32, tag="bcs")
                nc.vector.reciprocal_approx_fast(bcs[:], smb[:])
                at = at_pool.tile([HD, CH], BF16, tag="at",
                                  name=f"at{b}_{j}_{h}")
                nc.vector.tensor_tensor(at[:], pv[:], bcs[:],
                                        OP.mult)
                at_tiles.append(at)
                if prev is not None:
                    emit_fin_lt(prev[0], prev[1], prev[2], h)
            prev = (b, j, at_tiles)
        for lt in range(TPC):
            emit_fin_lt(prev[0], prev[1], prev[2], lt, tail=True)
    nc.compile()
    return nc


def host_tables(cfg):
    import ml_dtypes
    L, HD, theta = cfg["L"], cfg["HD"], cfg["theta"]
    half = HD // 2
    inv_freq = 1.0 / (theta ** (np.arange(half, dtype=np.float64) / half))
    ang = np.arange(L, dtype=np.float64)[:, None] * inv_freq[None, :]  # [L,half]
    cos = np.cos(ang).astype(np.float32)   # [L, half]
    sin = np.sin(ang).astype(np.float32)
    ropeA = np.empty((HD, L), dtype=np.float32)
    ropeB = np.empty((HD, L), dtype=np.float32)
    ropeA[:half] = cos.T
    ropeA[half:] = cos.T
    ropeB[:half] = -sin.T     # dest[:64] = cl[:64]*cos - cl[64:]*sin
    ropeB[half:] = sin.T      # dest[64:] = cl[64:]*cos + cl[:64]*sin

    perm = np.zeros((HD, HD), dtype=np.float32)
    for d in range(half):
        perm[d + half, d] = -1.0          # rot[d<64] = -cl[d+64]
    for d in range(half, HD):
        perm[d - half, d] = 1.0           # rot[d>=64] = +cl[d-64]

    # 0/1 lower-triangular validity mask for the diagonal block: [k, q]
    tri01 = (np.arange(HD)[None, :] >= np.arange(HD)[:, None]).astype(
        ml_dtypes.bfloat16)
    ones2 = np.ones((HD, HD), dtype=np.float32)
    ident = np.eye(HD, dtype=np.float32)
    return dict(ropeA=ropeA, ropeB=ropeB, tri01=tri01, ones2=ones2,
                ident=ident, perm=perm)


def host_in_maps(cfg, x, Wqkv, Wout):
    """Build per-core input maps from the full tensors."""
    B, L, D, QH, HD = cfg["B"], cfg["L"], cfg["D"], cfg["QH"], cfg["HD"]
    nco = cfg["ncores"]
    tabs = host_tables(cfg)
    import ml_dtypes
    xT = np.ascontiguousarray(
        x.reshape(B * L, D).T.astype(ml_dtypes.bfloat16))
    NHT = QH * nco      # total q heads
    in_maps = []
    for c in range(nco):
        q_rows = np.arange(c * QH * HD, (c + 1) * QH * HD)
        k_rows = np.arange(NHT * HD + c * HD, NHT * HD + (c + 1) * HD)
        v_rows = np.arange(NHT * HD + nco * HD + c * HD,
                           NHT * HD + nco * HD + (c + 1) * HD)
        rows = np.concatenate([q_rows, k_rows, v_rows])
        wq = np.ascontiguousarray(
            Wqkv[rows, :].T.astype(ml_dtypes.bfloat16))
        cols = np.arange(c * QH * HD, (c + 1) * QH * HD)
        wo = np.ascontiguousarray(Wout[:, cols].T.astype(ml_dtypes.bfloat16))
        m = dict(xT=xT, wq=wq, wo=wo)
        m.update(tabs)
        in_maps.append(m)
    return in_maps


_PROGRAM_CACHE = {}
LAST_RESULTS = None


def _get_program(cfg_key, cfg):
    if cfg_key not in _PROGRAM_CACHE:
        _PROGRAM_CACHE[cfg_key] = build_program(cfg)
    return _PROGRAM_CACHE[cfg_key]


def kernel(x, Wqkv, Wout):
    cfg = default_cfg()
    B, L, D = cfg["B"], cfg["L"], cfg["D"]
    x = np.asarray(x, dtype=np.float32)
    Wqkv = np.asarray(Wqkv, dtype=np.float32)
    Wout = np.asarray(Wout, dtype=np.float32)
    nc = _get_program("full", cfg)
    in_maps = host_in_maps(cfg, x, Wqkv, Wout)
    res = bass_utils.run_bass_kernel_spmd(
        nc, in_maps, core_ids=list(range(cfg["ncores"])))
    global LAST_RESULTS
    LAST_RESULTS = res
    parts = [res.results[c]["out"] for c in range(cfg["ncores"])]
    acc = np.zeros((B * L, D), dtype=np.float32)
    for p in parts:
        acc += p.astype(np.float32)
    return acc.reshape(B, L, D)


# ---------------------------------------------------------------------------
# dev helpers (not used by the grading harness)

def _np_partial_reference(cfg, x, Wqkv_rows, Wout_cols_T):
    """Numpy reference for ONE core's partial output.

    Wqkv_rows: [R, D] (q heads, k, v rows for this core)
    Wout_cols_T: [QH*HD, D] (transposed slice of Wout columns)
    """
    B, L, D, QH, HD = cfg["B"], cfg["L"], cfg["D"], cfg["QH"], cfg["HD"]
    CLIP, theta = cfg["CLIP"], cfg["theta"]
    half = HD // 2
    xf = x.reshape(B * L, D).astype(np.float64)
    qkv = np.clip(xf @ Wqkv_rows.astype(np.float64).T, -CLIP, CLIP)
    qkv = qkv.reshape(B, L, (QH + 2), HD)
    q = qkv[:, :, :QH, :]            # [B, L, QH, HD]
    k = qkv[:, :, QH, :]             # [B, L, HD]
    v = qkv[:, :, QH + 1, :]         # [B, L, HD]

    inv_freq = 1.0 / (theta ** (np.arange(half, dtype=np.float64) / half))
    ang = np.arange(L, dtype=np.float64)[:, None] * inv_freq[None, :]
    cos, sin = np.cos(ang), np.sin(ang)      # [L, half]

    def rope(t):  # t [B, L, ..., HD] with positions on axis 1
        t1, t2 = t[..., :half], t[..., half:]
        shape = [1, L] + [1] * (t.ndim - 3) + [half]
        c = cos.reshape(L, half).reshape(shape)
        s = sin.reshape(L, half).reshape(shape)
        return np.concatenate([t1 * c - t2 * s, t2 * c + t1 * s], axis=-1)

    q = rope(q)
    k = rope(k)
    scalev = 1.0 / math.sqrt(HD)
    causal = np.tril(np.ones((L, L), dtype=bool))
    outs = []
    for bi in range(B):
        heads = []
        for h in range(QH):
            s = (q[bi, :, h, :] @ k[bi].T) * scalev
            s = np.where(causal, s, -np.inf)
            p = np.exp(s - s.max(axis=-1, keepdims=True))
            p /= p.sum(axis=-1, keepdims=True)
            heads.append(p @ v[bi])
        attn = np.concatenate(heads, axis=-1)     # [L, QH*HD]
        outs.append(attn)
    attn = np.stack(outs, 0).reshape(B * L, QH * HD)
    return (attn @ Wout_cols_T.astype(np.float64)).astype(np.float32)


def _mini_test(mode="sim"):
    from concourse.bass_interp import CoreSim
    cfg = mini_cfg()
    B, L, D, QH, HD = cfg["B"], cfg["L"], cfg["D"], cfg["QH"], cfg["HD"]
    R = (QH + 2) * HD
    rng = np.random.default_rng(0)
    x = (rng.standard_normal((B, L, D)) * 1.0).astype(np.float32)
    Wqkv_rows = (rng.standard_normal((R, D)) * D ** -0.5).astype(np.float32)
    WoT = (rng.standard_normal((QH * HD, D)) * D ** -0.5).astype(np.float32)

    nc = build_program(cfg)
    tabs = host_tables(cfg)
    import ml_dtypes
    xT = np.ascontiguousarray(
        x.reshape(B * L, D).T.astype(ml_dtypes.bfloat16))
    wq = np.ascontiguousarray(Wqkv_rows.T.astype(ml_dtypes.bfloat16))
    in_map = dict(xT=xT, wq=wq, wo=WoT.astype(ml_dtypes.bfloat16))
    in_map.update(tabs)

    want = _np_partial_reference(cfg, x, Wqkv_rows, WoT)

    if mode == "sim":
        sim = CoreSim(nc, trace=False)
        for kk, vv in in_map.items():
            sim.tensor(kk)[:] = vv
        sim.simulate(check_with_hw=False)
        got = np.array(sim.tensor("out")).astype(np.float32)
    else:
        res = bass_utils.run_bass_kernel_spmd(nc, [in_map], core_ids=[0])
        got = res.results[0]["out"].astype(np.float32)
    relmax = np.abs(got - want).max() / np.abs(want).max()
    rel2 = np.linalg.norm(got - want) / np.linalg.norm(want)
    print(f"mini {mode}: relmax={relmax:.3e} rel2={rel2:.3e}")


if __name__ == "__main__":
    _mini_test(sys.argv[1] if len(sys.argv) > 1 else "sim")
